# revision 7
# baseline (speedup 1.0000x reference)
"""LayerNorm-LSTM Trainium2 kernel, 8-way tensor-parallel over the 4H gate dim.

Problem: T=64, B=256, NIN=1024, H=2048.
  per step: z = LN(x@wx)*gx+bx + LN(h@wh)*gh+bh + b   (LN over 4H)
            i,f,o = sigmoid(...), u = tanh(...)
            c = f*c + i*u ;  h = o*tanh(LN(c)*gc+bc)   (LN over H)
  with episode-reset mask applied to (c, h) at each step start.

Sharding: each of 8 cores owns H/8=256 rows of each gate (1024 of the 8192
gate columns, and a 256-wide slice of c/h), with the full batch B=256.
The x@wx contribution is precomputed for all T in a first phase (LN stats
shared via chunked AllReduce).  The sequential loop does the h@wh matmul
(M=128x2 full PE utilization, weights resident in SBUF as bf16), two tiny
stat AllReduces (LN over 4H / H couples the cores), and an AllGather of the
transposed h slices for the next step's matmul.
"""

import time

import numpy as np
import ml_dtypes

import os

import concourse.bass as bass
import concourse.mybir as mybir
import concourse.tile as tile
from concourse import bacc
from concourse.bass_utils import run_bass_kernel_spmd
from concourse.masks import make_identity

P = 128
NCORES = 8
EPS = 1e-5

F32 = mybir.dt.float32
BF16 = mybir.dt.float16  # fp16: 8x finer mantissa than bf16, same PE speed
ALU = mybir.AluOpType
AF = mybir.ActivationFunctionType


SKIP_COLL = bool(int(os.environ.get("KERNEL_SKIP_COLL", "0")))


def build(T, B, NIN, H, apply_gx, apply_bias, apply_gh, apply_gc, apply_bc):
    """Build the SPMD program (identical on all 8 cores; per-core data differs)."""
    HS = H // NCORES          # 256: H-slice per core
    LC = 4 * HS               # 1024: local gate columns per core
    KO = H // P               # 16 contraction chunks for h@wh
    KX = NIN // P             # 8 contraction chunks for x@wx
    NB = LC // 512            # 2 psum banks per row-tile
    MT = B // P               # 2 partition tiles of the batch
    RG = [list(range(NCORES))]

    nc = bacc.Bacc("TRN2", target_bir_lowering=False, debug=False,
                   num_devices=NCORES)

    # ---------------- DRAM I/O ----------------
    xT = nc.dram_tensor("xT", [NIN, T * B], BF16, kind="ExternalInput")
    wx_l = nc.dram_tensor("wx_l", [NIN, LC], BF16, kind="ExternalInput")
    wh_l = nc.dram_tensor("wh_l", [H, LC], BF16, kind="ExternalInput")
    bias_l = nc.dram_tensor("bias_l", [1, LC], F32, kind="ExternalInput")
    gx_l = nc.dram_tensor("gx_l", [1, LC], F32, kind="ExternalInput")
    gh_l = nc.dram_tensor("gh_l", [1, LC], F32, kind="ExternalInput")
    gc_l = nc.dram_tensor("gc_l", [1, HS], F32, kind="ExternalInput")
    bc_l = nc.dram_tensor("bc_l", [1, HS], F32, kind="ExternalInput")
    h0T = nc.dram_tensor("h0T", [H, B], BF16, kind="ExternalInput")
    c0_l = nc.dram_tensor("c0_l", [B, HS], F32, kind="ExternalInput")
    maskc = nc.dram_tensor("maskc", [B, T], F32, kind="ExternalInput")
    hs_l = nc.dram_tensor("hs_l", [T * B, HS], F32, kind="ExternalOutput")
    cf_l = nc.dram_tensor("cf_l", [B, HS], F32, kind="ExternalOutput")
    hf_l = nc.dram_tensor("hf_l", [B, HS], F32, kind="ExternalOutput")

    nchunks = T * B // P      # one chunk = 128 rows of the [T*B, LC] x-part
    SUP = 16                  # chunks per stats AllReduce super-chunk
    inv4h = 1.0 / (4 * H)
    invh = 1.0 / H

    with tile.TileContext(nc) as tc:
        with (
            tc.tile_pool(name="persist", bufs=1) as persist,
            tc.tile_pool(name="sbA", bufs=2) as sbA,
            tc.tile_pool(name="work", bufs=2) as work,
            tc.tile_pool(name="xwp", bufs=2 * SUP) as xwp,
            tc.tile_pool(name="stat", bufs=4) as stat,
            tc.tile_pool(name="psmm", bufs=4, space="PSUM") as psmm,
            tc.tile_pool(name="pstr", bufs=2, space="PSUM") as pstr,
            tc.tile_pool(name="dramp", bufs=1, space="DRAM") as dramp,
            tc.tile_pool(name="dramb", bufs=2, space="DRAM") as dramb,
        ):
            # ------------- persistent loads -------------
            whs = persist.tile([P, KO, LC], BF16)
            nc.sync.dma_start(whs, wh_l.rearrange("(ko p) n -> p ko n", p=P))
            wxs = persist.tile([P, KX, LC], BF16)
            nc.sync.dma_start(wxs, wx_l.rearrange("(ko p) n -> p ko n", p=P))
            masks = persist.tile([P, MT, T], F32)
            nc.sync.dma_start(masks, maskc.rearrange("(tl p) t -> p tl t", p=P))
            hT = persist.tile([P, KO, B], BF16)
            nc.sync.dma_start(hT, h0T.rearrange("(ko p) b -> p ko b", p=P))
            cst = persist.tile([P, MT, HS], F32)
            nc.sync.dma_start(cst, c0_l.rearrange("(tl p) f -> p tl f", p=P))
            ident = persist.tile([P, P], BF16)
            make_identity(nc, ident)
            eps1 = persist.tile([P, 1], F32)
            nc.vector.memset(eps1, EPS)
            if apply_bias:
                biass = persist.tile([1, LC], F32)
                nc.sync.dma_start(biass, bias_l)
            if apply_gx:
                gxs = persist.tile([1, LC], F32)
                nc.sync.dma_start(gxs, gx_l)
            if apply_gh:
                ghs = persist.tile([1, LC], F32)
                nc.sync.dma_start(ghs, gh_l)
            if apply_gc:
                gcs = persist.tile([1, HS], F32)
                nc.sync.dma_start(gcs, gc_l)
            if apply_bc:
                bcs = persist.tile([1, HS], F32)
                nc.sync.dma_start(bcs, bc_l)

            A_dram = dramp.tile([T, MT, P, LC], BF16)

            # ============ phase 1: A = LN(x@wx)*gx + (b+bx+bh) ============
            for sc0 in range(0, nchunks, SUP):
                cur = list(range(sc0, min(sc0 + SUP, nchunks)))
                ns = len(cur)
                pack = stat.tile([P, 2 * SUP], F32, tag="ppack")
                xw_tiles = []
                for idx, ci in enumerate(cur):
                    xTs = work.tile([P, KX, P], BF16, tag="xTs")
                    nc.sync.dma_start(
                        xTs, xT[:, ci * P:(ci + 1) * P]
                        .rearrange("(ko p) r -> p ko r", p=P))
                    pss = []
                    for n in range(NB):
                        ps = psmm.tile([P, 512], F32, tag="mm512")
                        for k in range(KX):
                            nc.tensor.matmul(
                                ps, xTs[:, k], wxs[:, k, n * 512:(n + 1) * 512],
                                start=(k == 0), stop=(k == KX - 1))
                        pss.append(ps)
                    st6 = stat.tile([P, NB, 6], F32, tag="st6")
                    for n in range(NB):
                        nc.vector.bn_stats(st6[:, n], pss[n])
                    mv = stat.tile([P, 2], F32, tag="mv")
                    nc.vector.bn_aggr(mv, st6)
                    # pack local (sum, sumsq) scaled by LC
                    musq = stat.tile([P, 1], F32, tag="musq")
                    nc.vector.tensor_mul(musq, mv[:, 0:1], mv[:, 0:1])
                    q = stat.tile([P, 1], F32, tag="q")
                    nc.vector.tensor_add(q, mv[:, 1:2], musq)
                    nc.vector.tensor_scalar_mul(
                        pack[:, idx:idx + 1], mv[:, 0:1], float(LC))
                    nc.vector.tensor_scalar_mul(
                        pack[:, SUP + idx:SUP + idx + 1], q, float(LC))
                    xwsb = xwp.tile([P, LC], BF16, tag="xwsb")
                    for n in range(NB):
                        nc.vector.tensor_copy(
                            xwsb[:, n * 512:(n + 1) * 512], pss[n])
                    xw_tiles.append(xwsb)
                bin_ = dramb.tile([P, 2 * SUP], F32, tag="binp")
                bout = dramb.tile([P, 2 * SUP], F32, tag="boutp",
                                  addr_space="Shared")
                nc.gpsimd.dma_start(bin_, pack)
                if SKIP_COLL:
                    nc.gpsimd.dma_start(bout, bin_)
                else:
                    nc.gpsimd.collective_compute(
                        "AllReduce", ALU.add, replica_groups=RG,
                        ins=[bin_[:].opt()], outs=[bout[:].opt()])
                tot = stat.tile([P, 2 * SUP], F32, tag="totp")
                nc.gpsimd.dma_start(tot, bout)
                mom = stat.tile([P, 2 * SUP], F32, tag="momp")
                nc.vector.tensor_scalar_mul(mom, tot, inv4h)
                mus = mom[:, 0:SUP]
                e2s = mom[:, SUP:2 * SUP]
                msq = stat.tile([P, SUP], F32, tag="msqp")
                nc.vector.tensor_mul(msq, mus, mus)
                var = stat.tile([P, SUP], F32, tag="varp")
                nc.vector.tensor_tensor(var, e2s, msq, ALU.subtract)
                sd = stat.tile([P, SUP], F32, tag="sdp")
                nc.scalar.activation(sd, var, AF.Sqrt, bias=eps1)
                rstd = stat.tile([P, SUP], F32, tag="rstdp")
                nc.vector.reciprocal(rstd, sd)
                for idx, ci in enumerate(cur):
                    t_i, tl = divmod(ci, MT)
                    a_out = work.tile([P, LC], BF16, tag="aout")
                    nc.vector.tensor_scalar(
                        a_out, xw_tiles[idx],
                        mus[:, idx:idx + 1], rstd[:, idx:idx + 1],
                        op0=ALU.subtract, op1=ALU.mult)
                    if apply_gx:
                        nc.vector.tensor_mul(
                            a_out, a_out, gxs.to_broadcast([P, LC]))
                    if apply_bias:
                        nc.vector.tensor_add(
                            a_out, a_out, biass.to_broadcast([P, LC]))
                    nc.sync.dma_start(A_dram[t_i, tl], a_out)

            # ============ phase 2: the recurrence ============
            for t in range(T):
                At = sbA.tile([P, MT, LC], BF16, tag="At")
                nc.sync.dma_start(At, A_dram[t].rearrange("tl p n -> p tl n"))

                # hw = h @ wh  (local gate columns, full batch)
                pss = [[None] * NB for _ in range(MT)]
                for m in range(MT):
                    for n in range(NB):
                        ps = psmm.tile([P, 512], F32, tag="mm512")
                        for k in range(KO):
                            nc.tensor.matmul(
                                ps, hT[:, k, m * P:(m + 1) * P],
                                whs[:, k, n * 512:(n + 1) * 512],
                                start=(k == 0), stop=(k == KO - 1))
                        pss[m][n] = ps

                # local LN partials over the 1024 local columns
                pack = stat.tile([P, 4], F32, tag="rpack")
                for m in range(MT):
                    st6 = stat.tile([P, NB, 6], F32, tag="rst6")
                    for n in range(NB):
                        nc.vector.bn_stats(st6[:, n], pss[m][n])
                    mv = stat.tile([P, 2], F32, tag="rmv")
                    nc.vector.bn_aggr(mv, st6)
                    musq = stat.tile([P, 1], F32, tag="rmusq")
                    nc.vector.tensor_mul(musq, mv[:, 0:1], mv[:, 0:1])
                    q = stat.tile([P, 1], F32, tag="rq")
                    nc.vector.tensor_add(q, mv[:, 1:2], musq)
                    nc.vector.tensor_scalar_mul(
                        pack[:, m:m + 1], mv[:, 0:1], float(LC))
                    nc.vector.tensor_scalar_mul(
                        pack[:, 2 + m:3 + m], q, float(LC))
                bin1 = dramb.tile([P, 4], F32, tag="bin1")
                bout1 = dramb.tile([P, 4], F32, tag="bout1",
                                   addr_space="Shared")
                nc.gpsimd.dma_start(bin1, pack)
                if SKIP_COLL:
                    nc.gpsimd.dma_start(bout1, bin1)
                else:
                    nc.gpsimd.collective_compute(
                        "AllReduce", ALU.add, replica_groups=RG,
                        ins=[bin1[:].opt()], outs=[bout1[:].opt()])
                tot1 = stat.tile([P, 4], F32, tag="tot1")
                nc.gpsimd.dma_start(tot1, bout1)
                mom1 = stat.tile([P, 4], F32, tag="mom1")
                nc.vector.tensor_scalar_mul(mom1, tot1, inv4h)
                mu1 = mom1[:, 0:MT]
                msq1 = stat.tile([P, MT], F32, tag="msq1")
                nc.vector.tensor_mul(msq1, mu1, mu1)
                var1 = stat.tile([P, MT], F32, tag="var1")
                nc.vector.tensor_tensor(var1, mom1[:, MT:2 * MT], msq1,
                                        ALU.subtract)
                sd1 = stat.tile([P, MT], F32, tag="sd1")
                nc.scalar.activation(sd1, var1, AF.Sqrt, bias=eps1)
                rstd1 = stat.tile([P, MT], F32, tag="rstd1")
                nc.vector.reciprocal(rstd1, sd1)

                # z = LN(hw)[*gh] + A_t ; gates
                g = work.tile([P, MT, LC], BF16, tag="g")
                for m in range(MT):
                    z = work.tile([P, LC], F32, tag="z")
                    for n in range(NB):
                        nc.vector.tensor_scalar(
                            z[:, n * 512:(n + 1) * 512], pss[m][n],
                            mu1[:, m:m + 1], rstd1[:, m:m + 1],
                            op0=ALU.subtract, op1=ALU.mult)
                    if apply_gh:
                        nc.vector.tensor_mul(z, z, ghs.to_broadcast([P, LC]))
                    nc.vector.tensor_add(z, z, At[:, m])
                    nc.scalar.activation(g[:, m, 0:3 * HS], z[:, 0:3 * HS],
                                         AF.Sigmoid)
                    nc.scalar.activation(g[:, m, 3 * HS:], z[:, 3 * HS:],
                                         AF.Tanh)

                # c = f * (c * keep_t) + i * u ; local LN(c) partials
                pack2 = stat.tile([P, 2 * MT], F32, tag="pack2")
                for m in range(MT):
                    nc.vector.tensor_scalar_mul(
                        cst[:, m], cst[:, m], masks[:, m, t:t + 1])
                    iu = work.tile([P, HS], BF16, tag="iu")
                    nc.vector.tensor_mul(iu, g[:, m, 0:HS],
                                         g[:, m, 3 * HS:4 * HS])
                    fc = work.tile([P, HS], F32, tag="fc")
                    nc.vector.tensor_mul(fc, g[:, m, HS:2 * HS], cst[:, m])
                    nc.vector.tensor_add(cst[:, m], fc, iu)
                    st1 = stat.tile([P, 6], F32, tag="st1")
                    nc.vector.bn_stats(st1, cst[:, m])
                    mv2 = stat.tile([P, 2], F32, tag="mv2")
                    nc.vector.bn_aggr(mv2, st1)
                    musq2 = stat.tile([P, 1], F32, tag="musq2")
                    nc.vector.tensor_mul(musq2, mv2[:, 0:1], mv2[:, 0:1])
                    q2 = stat.tile([P, 1], F32, tag="q2")
                    nc.vector.tensor_add(q2, mv2[:, 1:2], musq2)
                    nc.vector.tensor_scalar_mul(
                        pack2[:, m:m + 1], mv2[:, 0:1], float(HS))
                    nc.vector.tensor_scalar_mul(
                        pack2[:, MT + m:MT + m + 1], q2, float(HS))
                bin2 = dramb.tile([P, 2 * MT], F32, tag="bin2")
                bout2 = dramb.tile([P, 2 * MT], F32, tag="bout2",
                                   addr_space="Shared")
                nc.gpsimd.dma_start(bin2, pack2)
                if SKIP_COLL:
                    nc.gpsimd.dma_start(bout2, bin2)
                else:
                    nc.gpsimd.collective_compute(
                        "AllReduce", ALU.add, replica_groups=RG,
                        ins=[bin2[:].opt()], outs=[bout2[:].opt()])
                tot2 = stat.tile([P, 2 * MT], F32, tag="tot2")
                nc.gpsimd.dma_start(tot2, bout2)
                mom2 = stat.tile([P, 2 * MT], F32, tag="mom2")
                nc.vector.tensor_scalar_mul(mom2, tot2, invh)
                mu2 = mom2[:, 0:MT]
                msq3 = stat.tile([P, MT], F32, tag="msq3")
                nc.vector.tensor_mul(msq3, mu2, mu2)
                var2 = stat.tile([P, MT], F32, tag="var2")
                nc.vector.tensor_tensor(var2, mom2[:, MT:2 * MT], msq3,
                                        ALU.subtract)
                sd2 = stat.tile([P, MT], F32, tag="sd2")
                nc.scalar.activation(sd2, var2, AF.Sqrt, bias=eps1)
                rstd2 = stat.tile([P, MT], F32, tag="rstd2")
                nc.vector.reciprocal(rstd2, sd2)

                # h = o * tanh(LN(c)[*gc][+bc])
                h = work.tile([P, MT, HS], F32, tag="h")
                for m in range(MT):
                    tl_ = work.tile([P, HS], F32, tag="tl_")
                    nc.vector.tensor_scalar(
                        tl_, cst[:, m], mu2[:, m:m + 1], rstd2[:, m:m + 1],
                        op0=ALU.subtract, op1=ALU.mult)
                    if apply_gc:
                        nc.vector.tensor_mul(tl_, tl_,
                                             gcs.to_broadcast([P, HS]))
                    if apply_bc:
                        nc.vector.tensor_add(tl_, tl_,
                                             bcs.to_broadcast([P, HS]))
                    th = work.tile([P, HS], BF16, tag="th")
                    nc.scalar.activation(th, tl_, AF.Tanh)
                    nc.vector.tensor_mul(h[:, m], g[:, m, 2 * HS:3 * HS], th)

                nc.sync.dma_start(
                    hs_l[t * B:(t + 1) * B].rearrange("(tl p) f -> p tl f",
                                                      p=P), h)
                if t == T - 1:
                    nc.sync.dma_start(
                        cf_l.rearrange("(tl p) f -> p tl f", p=P), cst)
                    nc.sync.dma_start(
                        hf_l.rearrange("(tl p) f -> p tl f", p=P), h)
                    continue

                # mask for next step, transpose, all-gather hT
                hm = work.tile([P, MT, HS], BF16, tag="hm")
                for m in range(MT):
                    nc.vector.tensor_scalar_mul(
                        hm[:, m], h[:, m], masks[:, m, t + 1:t + 2])
                hTmine = work.tile([P, HS // P, B], BF16, tag="hTmine")
                for m in range(MT):
                    for fo in range(HS // P):
                        pst = pstr.tile([P, P], BF16, tag="ptr")
                        nc.tensor.transpose(
                            pst, hm[:, m, fo * P:(fo + 1) * P], ident)
                        nc.vector.tensor_copy(
                            hTmine[:, fo, m * P:(m + 1) * P], pst)
                bin3 = dramb.tile([P, HS // P, B], BF16, tag="bin3")
                bout3 = dramb.tile([NCORES, P, HS // P, B], BF16, tag="bout3",
                                   addr_space="Shared")
                nc.gpsimd.dma_start(bin3, hTmine)
                if SKIP_COLL:
                    for _r in range(NCORES):
                        nc.gpsimd.dma_start(bout3[_r], bin3)
                else:
                    nc.gpsimd.collective_compute(
                        "AllGather", ALU.bypass, replica_groups=RG,
                        ins=[bin3[:].opt()], outs=[bout3[:].opt()])
                nc.gpsimd.dma_start(
                    hT.rearrange("p (r k) b -> p r k b", r=NCORES),
                    bout3.rearrange("r p k b -> p r k b"))

    nc.compile()
    return nc


_NC_CACHE = {}
LAST_EXEC_S = None


def _get_nc(key, *args):
    if key not in _NC_CACHE:
        _NC_CACHE[key] = build(*args)
    return _NC_CACHE[key]


def kernel(x, mask, initial_state, wx, wh, b, gx, bx, gh, bh, gc, bc):
    x = np.asarray(x, dtype=np.float32)
    mask = np.asarray(mask, dtype=np.float32)
    initial_state = np.asarray(initial_state, dtype=np.float32)
    wx = np.asarray(wx, dtype=np.float32)
    wh = np.asarray(wh, dtype=np.float32)
    T, B, NIN = x.shape
    H = wh.shape[0]
    HS = H // NCORES
    LC = 4 * HS

    b = np.asarray(b, dtype=np.float32)
    gx = np.asarray(gx, dtype=np.float32)
    bx = np.asarray(bx, dtype=np.float32)
    gh = np.asarray(gh, dtype=np.float32)
    bh = np.asarray(bh, dtype=np.float32)
    gc = np.asarray(gc, dtype=np.float32)
    bc = np.asarray(bc, dtype=np.float32)

    apply_gx = not np.all(gx == 1.0)
    apply_gh = not np.all(gh == 1.0)
    apply_gc = not np.all(gc == 1.0)
    apply_bc = not np.all(bc == 0.0)
    bias = b + bx + bh
    apply_bias = not np.all(bias == 0.0)

    nc = _get_nc((T, B, NIN, H, apply_gx, apply_bias, apply_gh, apply_gc,
                  apply_bc, SKIP_COLL),
                 T, B, NIN, H, apply_gx, apply_bias, apply_gh, apply_gc,
                 apply_bc)

    # ---- host-side input marshalling ----
    bf = np.float16
    xT = np.ascontiguousarray(
        x.reshape(T * B, NIN).T).astype(bf)                       # [NIN, T*B]
    keep = (1.0 - mask[:, :, 0])                                  # [T, B]
    maskc = np.ascontiguousarray(keep.T).astype(np.float32)       # [B, T]
    s0 = initial_state.reshape(B, T, 2 * H)[:, 0]
    c0 = s0[:, :H]
    h0 = s0[:, H:]
    h0m = h0 * keep[0][:, None]                                   # mask step 0
    h0T = np.ascontiguousarray(h0m.T).astype(bf)                  # [H, B]

    # per-core gate-column slices: core g owns rows [g*HS,(g+1)*HS) of each gate
    def gate_cols(g):
        return np.concatenate(
            [np.arange(k * H + g * HS, k * H + (g + 1) * HS) for k in range(4)])

    in_maps = []
    for g in range(NCORES):
        cols = gate_cols(g)
        m = {
            "xT": xT,
            "wx_l": np.ascontiguousarray(wx[:, cols]).astype(bf),
            "wh_l": np.ascontiguousarray(wh[:, cols]).astype(bf),
            "bias_l": np.ascontiguousarray(bias[cols])[None, :],
            "gx_l": np.ascontiguousarray(gx[cols])[None, :],
            "gh_l": np.ascontiguousarray(gh[cols])[None, :],
            "gc_l": np.ascontiguousarray(gc[g * HS:(g + 1) * HS])[None, :],
            "bc_l": np.ascontiguousarray(bc[g * HS:(g + 1) * HS])[None, :],
            "h0T": h0T,
            "c0_l": np.ascontiguousarray(c0[:, g * HS:(g + 1) * HS]),
            "maskc": maskc,
        }
        in_maps.append(m)

    global LAST_EXEC_S
    t0 = time.time()
    res = run_bass_kernel_spmd(nc, in_maps, core_ids=list(range(NCORES)))
    LAST_EXEC_S = time.time() - t0

    hs = np.empty((T, B, H), dtype=np.float32)
    s = np.empty((B, 2 * H), dtype=np.float32)
    for g in range(NCORES):
        r = res.results[g]
        hs[:, :, g * HS:(g + 1) * HS] = r["hs_l"].reshape(T, B, HS)
        s[:, g * HS:(g + 1) * HS] = r["cf_l"]
        s[:, H + g * HS:H + (g + 1) * HS] = r["hf_l"]
    return hs, s
